# revision 6
# baseline (speedup 1.0000x reference)
"""Fused single-head attention (QKV proj + softmax*scale + AV) on 8 trn2 cores.

Reference computation (fp32):
    qkv = x @ W.T            x:[4,4096,768]  W:[192,768]
    q,k,v = split(qkv, 64)
    A = q @ k.T              (no pre-softmax scale)
    out = softmax(A) / 8 @ v

Sharding: core c handles batch b=c//2, query half qh=c%2 (2048 queries),
full 4096 keys of that batch. SPMD-uniform program: the host rolls the
key/value columns of x^T by qh*2048 so every core's own queries are
always columns 0:2048 (softmax is permutation-invariant over keys).

Device layout (per core):
    xt  [768, 4096] = x[b].T rolled     (f32, bitcast to f32r for matmuls)
    wt  [768, 192]  = W[perm].T, perm = [K rows | V rows | Q rows]
    projection -> qkv^T tiles in PSUM: [K^T|V^T] (M=128) and Q^T (M=64)
    scores A^T[k,q] = (K^T tile).T @ Q^T   (f32r, contraction dh=64)
    P^T = exp(A^T - 40) in bf16 (no row max needed: |A| <= ~77)
    out^T[dh,q] (+rowsum in row 64) accumulated over k-tiles:
        lhsT = V_aug [128, 65] (col 64 = ones), rhs = P^T chunk
    out[q,dh] = PE-transpose(out^T) / (8 * rowsum)
"""

import sys

import numpy as np

for _p in ("/opt/trn_rl_repo",):
    if _p not in sys.path:
        sys.path.insert(0, _p)

import concourse.bass as bass  # noqa: E402
import concourse.mybir as mybir  # noqa: E402
import concourse.tile as tile  # noqa: E402
from concourse import bacc  # noqa: E402
from concourse.bass_utils import run_bass_kernel_spmd  # noqa: E402
from concourse.masks import make_identity  # noqa: E402

B, S, D, DH = 4, 4096, 768, 64
QN = S // 2          # queries per core
NSB = 8              # 512-wide super-blocks of s
NKT = 32             # 128-wide key tiles
HALF = 1024          # q-chunk for the main loop
EXP_BIAS = -40.0     # global score offset (softmax-invariant), fp32 headroom

F32 = mybir.dt.float32
F32R = mybir.dt.float32r
BF16 = mybir.dt.bfloat16

_NC_CACHE = None
LAST_RESULTS = None


def _build():
    nc = bacc.Bacc(num_devices=8)
    xt_d = nc.dram_tensor("xt", [D, S], F32R, kind="ExternalInput")
    wt_d = nc.dram_tensor("wt", [D, 3 * DH], F32R, kind="ExternalInput")
    out_d = nc.dram_tensor("out", [QN, DH], F32, kind="ExternalOutput")

    with tile.TileContext(nc) as tc:
        with (
            tc.tile_pool(name="big", bufs=1) as big,
            tc.tile_pool(name="psmm", bufs=2, space="PSUM") as psmm,
            tc.tile_pool(name="psacc", bufs=1, space="PSUM") as psacc,
            tc.tile_pool(name="pt", bufs=3) as ptp,
            tc.tile_pool(name="small", bufs=4) as small,
        ):
            xt_sb = big.tile([128, 6, S], F32R)
            wt_sb = big.tile([128, 6, 3 * DH], F32R)
            kt_sb = big.tile([64, S], F32R)
            qt_sb = big.tile([64, QN], F32R)
            vt_sb = big.tile([64, S], BF16)
            v_sb = big.tile([128, NKT, 80], BF16)   # [...,0:64]=V tile, 64=ones
            acc_sb = big.tile([65, QN], F32)
            osb = big.tile([128, 16, DH], F32)
            ident = big.tile([128, 128], F32)
            identb = big.tile([128, 128], BF16)

            make_identity(nc, ident[:])
            make_identity(nc, identb[:])
            nc.vector.memset(v_sb[:, :, 64:65], 1.0)
            ebias = big.tile([128, 1], F32)
            nc.vector.memset(ebias[:], EXP_BIAS)

            nc.sync.dma_start(
                out=wt_sb[:], in_=wt_d[:].rearrange("(k p) e -> p k e", p=128)
            )

            # ---- projection over 8 super-blocks (own q-half first: sb 0..3)
            for sb in range(NSB):
                sl = slice(sb * 512, (sb + 1) * 512)
                for k in range(6):
                    nc.sync.dma_start(
                        out=xt_sb[:, k, sl],
                        in_=xt_d[k * 128:(k + 1) * 128, sl],
                    )
                kv_ps = psmm.tile([128, HALF], F32, tag="mm")
                for k in range(6):
                    nc.tensor.matmul(
                        kv_ps[:, 0:512],
                        wt_sb[:, k, 0:128],
                        xt_sb[:, k, sl],
                        start=(k == 0),
                        stop=(k == 5),
                    )
                nc.vector.tensor_copy(kt_sb[:, sl], kv_ps[0:64, 0:512])
                nc.vector.tensor_copy(vt_sb[:, sl], kv_ps[64:128, 0:512])
                if sb < 4:
                    q_ps = psmm.tile([64, HALF], F32, tag="mm")
                    for k in range(6):
                        nc.tensor.matmul(
                            q_ps[:, 0:512],
                            wt_sb[:, k, 128:192],
                            xt_sb[:, k, sl],
                            start=(k == 0),
                            stop=(k == 5),
                        )
                    nc.vector.tensor_copy(qt_sb[:, sl], q_ps[:, 0:512])
                # V natural tiles via PE transpose, 4 k-tiles per sb
                for t4 in range(4):
                    t = sb * 4 + t4
                    vtp = psmm.tile([128, HALF], BF16, tag="vtp")
                    nc.tensor.transpose(
                        vtp[:, 0:64],
                        vt_sb[:, t * 128:(t + 1) * 128],
                        identb[0:64, 0:64],
                    )
                    nc.vector.tensor_copy(v_sb[:, t, 0:64], vtp[:, 0:64])

            # ---- main loop: q in halves of 1024, flash accumulate over k
            for h in range(2):
                qsl = slice(h * HALF, (h + 1) * HALF)
                acc = psacc.tile([65, HALF], F32, tag="acc")
                for t in range(NKT):
                    at = psmm.tile([128, HALF], F32, tag="mm")
                    for g in range(2):
                        gsl = slice(h * HALF + g * 512, h * HALF + g * 512 + 512)
                        nc.tensor.matmul(
                            at[:, g * 512:(g + 1) * 512],
                            kt_sb[:, t * 128:(t + 1) * 128],
                            qt_sb[:, gsl],
                            start=True,
                            stop=True,
                        )
                    pt = ptp.tile([128, HALF], BF16, tag="pt")
                    nc.scalar.activation(
                        out=pt[:],
                        in_=at[:],
                        func=mybir.ActivationFunctionType.Exp,
                        bias=ebias[:],
                    )
                    for g in range(2):
                        nc.tensor.matmul(
                            acc[:, g * 512:(g + 1) * 512],
                            v_sb[:, t, 0:65],
                            pt[:, g * 512:(g + 1) * 512],
                            start=(t == 0),
                            stop=(t == NKT - 1),
                            skip_group_check=True,
                        )
                nc.vector.tensor_copy(acc_sb[:, qsl], acc[:])

                # finalize this half: transpose + normalize
                for blk in range(8):
                    gblk = h * 8 + blk
                    ot = psmm.tile([128, HALF], F32, tag="mm")
                    nc.tensor.transpose(
                        ot[:, 0:65],
                        acc_sb[:, gblk * 128:(gblk + 1) * 128],
                        ident[0:65, 0:65],
                    )
                    r = small.tile([128, 1], F32, tag="r")
                    nc.vector.tensor_scalar_mul(r[:], ot[:, 64:65], 8.0)
                    nc.vector.reciprocal(r[:], r[:])
                    nc.vector.tensor_scalar_mul(osb[:, gblk, :], ot[:, 0:64], r[:])

            nc.sync.dma_start(
                out=out_d[:].rearrange("(t p) d -> p t d", p=128), in_=osb[:]
            )

    nc.finalize()
    return nc


def _get_nc():
    global _NC_CACHE
    if _NC_CACHE is None:
        _NC_CACHE = _build()
    return _NC_CACHE


def kernel(x, W, _trace=False):
    global LAST_RESULTS
    x = np.ascontiguousarray(np.asarray(x), dtype=np.float32)
    W = np.ascontiguousarray(np.asarray(W), dtype=np.float32)
    assert x.shape == (B, S, D) and W.shape == (3 * DH, D)

    # wt columns: [K | V | Q] so proj M-tile0 = [K^T|V^T], M-tile1 = Q^T
    wt = np.ascontiguousarray(
        np.concatenate([W[DH:2 * DH], W[2 * DH:], W[:DH]], axis=0).T
    )

    in_maps = []
    for c in range(8):
        b, qh = divmod(c, 2)
        xtb = x[b].T  # [768, 4096]
        if qh:
            xtc = np.ascontiguousarray(
                np.concatenate([xtb[:, QN:], xtb[:, :QN]], axis=1)
            )
        else:
            xtc = np.ascontiguousarray(xtb)
        in_maps.append({"xt": xtc, "wt": wt})

    nc = _get_nc()
    res = run_bass_kernel_spmd(nc, in_maps, list(range(8)), trace=_trace)
    LAST_RESULTS = res

    out = np.empty((B, S, DH), np.float32)
    for c in range(8):
        b, qh = divmod(c, 2)
        out[b, qh * QN:(qh + 1) * QN] = res.results[c]["out"]
    return out
